# revision 1
# baseline (speedup 1.0000x reference)
# Trainium2 Bass kernel for nn_EpisodeMultiheadAttentionBlock.
# B=8, S=1024, E=1024, H=8 heads, HD=128. Data-parallel over batch: core b
# computes batch element b. Self-contained: only needs /opt/trn_rl_repo on path.
import sys
import numpy as np

sys.path.insert(0, "/opt/trn_rl_repo")

import ml_dtypes  # noqa: E402
import concourse.bass as bass  # noqa: E402
import concourse.mybir as mybir  # noqa: E402
import concourse.tile as tile  # noqa: E402
from concourse import bacc  # noqa: E402
from concourse import bass_utils  # noqa: E402

B, S, E, H = 8, 1024, 1024, 8
HD = E // H  # 128
NT = E // 128  # 8 e-tiles / s-tiles
NC = 8  # cores
BF16 = mybir.dt.bfloat16
F32 = mybir.dt.float32
I16 = mybir.dt.int16
AF = mybir.ActivationFunctionType
NPBF16 = ml_dtypes.bfloat16

_COMPILED = {}
EXP = {}


def _build(share_qk: bool, reps: int = 1, phase_limit: int = 99):
    """Build + compile the per-core Bass program. share_qk: query_index==key_index."""
    nc = bacc.Bacc("TRN2", target_bir_lowering=False, debug=False, num_devices=NC)

    # ---- DRAM tensors -------------------------------------------------------
    x_d = nc.dram_tensor("x", [S, E], F32, kind="ExternalInput")
    w_d = {
        nm: nc.dram_tensor(nm, [128, NT * E], BF16, kind="ExternalInput")
        for nm in ("Wq", "Wk", "Wv", "Wo", "Wxr", "Wyr", "Wxz", "Wyz", "Wxg", "Wyg")
    }
    bq_d = nc.dram_tensor("bq", [128, NT], F32, kind="ExternalInput")
    bk_d = nc.dram_tensor("bk", [128, NT], F32, kind="ExternalInput")
    bo_d = nc.dram_tensor("bo", [128, NT], F32, kind="ExternalInput")
    bv_row_d = nc.dram_tensor("bv_row", [1, E], BF16, kind="ExternalInput")
    bxz_row_d = nc.dram_tensor("bxz_row", [1, E], BF16, kind="ExternalInput")
    invs_d = nc.dram_tensor("invp_sin", [2, NT * 128], F32, kind="ExternalInput")
    invc_d = nc.dram_tensor("invp_cos", [2, NT * 128], F32, kind="ExternalInput")
    idxfq_d = nc.dram_tensor("idxf_q", [2, S], F32, kind="ExternalInput")
    if not share_qk:
        idxfk_d = nc.dram_tensor("idxf_k", [2, S], F32, kind="ExternalInput")
    ident_d = nc.dram_tensor("ident", [128, 128], BF16, kind="ExternalInput")
    pmat_d = nc.dram_tensor("pmat", [128, 128], BF16, kind="ExternalInput")
    out_d = nc.dram_tensor("out", [S, E], F32, kind="ExternalOutput")

    SCALE = 1.0 / float(np.sqrt(HD))

    with tile.TileContext(nc) as tc:
      from contextlib import ExitStack

      for _rep in range(reps):
        with ExitStack() as top:
            # ---------------- resident pools --------------------------------
            res = top.enter_context(tc.tile_pool(name="res", bufs=1))
            xT = res.tile([128, NT, S], BF16, tag="xT")      # x^T  [e, s]
            ctxT = res.tile([128, NT, S], BF16, tag="ctxT")  # attn out^T (normalized)

            consts = top.enter_context(tc.tile_pool(name="consts", bufs=1))
            ident = consts.tile([128, 128], BF16, tag="ident")
            nc.sync.dma_start(out=ident, in_=ident_d.ap())
            pmat = consts.tile([128, 128], BF16, tag="pmat")
            nc.sync.dma_start(out=pmat, in_=pmat_d.ap())
            bq_sb = consts.tile([128, NT], F32, tag="bq")
            nc.sync.dma_start(out=bq_sb, in_=bq_d.ap())
            bk_sb = consts.tile([128, NT], F32, tag="bk")
            nc.sync.dma_start(out=bk_sb, in_=bk_d.ap())
            bo_sb = consts.tile([128, NT], F32, tag="bo")
            nc.sync.dma_start(out=bo_sb, in_=bo_d.ap())
            bv_row = consts.tile([1, E], BF16, tag="bv_row")
            nc.sync.dma_start(out=bv_row, in_=bv_row_d.ap())
            bxz_row = consts.tile([1, E], BF16, tag="bxz_row")
            nc.sync.dma_start(out=bxz_row, in_=bxz_row_d.ap())
            ones_den = consts.tile([128, 1], BF16, tag="ones_den")
            nc.vector.memset(ones_den, 1.0)
            ones1_b = consts.tile([1, 128], BF16, tag="ones1_b")
            nc.vector.memset(ones1_b, 1.0)
            ones1_f = consts.tile([1, 128], F32, tag="ones1_f")
            nc.vector.memset(ones1_f, 1.0)
            invs_sb = consts.tile([2, NT * 128], F32, tag="invs")
            nc.sync.dma_start(out=invs_sb, in_=invs_d.ap())
            invc_sb = consts.tile([2, NT * 128], F32, tag="invc")
            nc.sync.dma_start(out=invc_sb, in_=invc_d.ap())
            idxfq_sb = consts.tile([2, S], F32, tag="idxfq")
            nc.sync.dma_start(out=idxfq_sb, in_=idxfq_d.ap())
            if not share_qk:
                idxfk_sb = consts.tile([2, S], F32, tag="idxfk")
                nc.sync.dma_start(out=idxfk_sb, in_=idxfk_d.ap())
            else:
                idxfk_sb = idxfq_sb

            # weight streaming pool (each slot holds one full packed matrix)
            wpool = top.enter_context(tc.tile_pool(name="wpool", bufs=EXP.get("wbufs", 3)))

            # vsb/qr/kr live only through P4 (pool closed before P5)
            mid_ctx = tc.tile_pool(name="mid", bufs=1)
            mid = mid_ctx.__enter__()
            vsb = mid.tile([128, NT, E], BF16, tag="vsb")    # v    [s, e]
            qr = mid.tile([128, NT, S], BF16, tag="qr")      # rope(q)^T
            kr = mid.tile([128, NT, S], BF16, tag="kr")      # rope(k)^T

            def load_w(nm):
                t = wpool.tile([128, NT, E], BF16, tag="W")
                nc.sync.dma_start(out=t, in_=w_d[nm].ap().rearrange("p (t e) -> p t e", t=NT))
                return t

            # =========== P1: load x (cast bf16) and transpose -> xT ==========
            with tc.tile_pool(name="p1", bufs=3) as p1, \
                 tc.tile_pool(name="p1ps", bufs=4, space="PSUM") as p1ps:
                wv_sb = load_w("Wv")  # prefetch Wv during transpose phase
                wq_sb = load_w("Wq")
                for st in range(NT):
                    xb = p1.tile([128, E], BF16, tag="xb")
                    nc.gpsimd.dma_start(out=xb, in_=x_d.ap()[st * 128:(st + 1) * 128, :])
                    for et in range(NT):
                        tp = p1ps.tile([128, 128], BF16, tag="tp")
                        nc.tensor.transpose(tp, xb[:, et * 128:(et + 1) * 128], ident)
                        nc.vector.tensor_copy(xT[:, et, st * 128:(st + 1) * 128], tp)

                # =========== P2: v = x @ Wv + bv   (seq-major) ===============
                for st in range(NT):
                    for c in range(2):
                        ps = p1ps.tile([128, 512], F32, tag="mm")
                        for kt in range(NT):
                            nc.tensor.matmul(
                                ps,
                                lhsT=xT[:, kt, st * 128:(st + 1) * 128],
                                rhs=wv_sb[:, kt, c * 512:(c + 1) * 512],
                                start=(kt == 0), stop=False)
                        nc.tensor.matmul(
                            ps, lhsT=ones1_b, rhs=bv_row[:, c * 512:(c + 1) * 512],
                            start=False, stop=True)
                        nc.vector.tensor_copy(vsb[:, st, c * 512:(c + 1) * 512], ps)

            if phase_limit < 3:
                nc.sync.dma_start(out=out_d.ap(), in_=x_d.ap())
                continue
            # =========== P3: q/k proj + RoPE (per e-tile) ====================
            # cos/sin computed on device: f = inv[e] * idx[s] (+ pi/2 for cos)
            # via a K=2 f32 outer-product matmul, range-reduced with
            # k = round(f/2pi) (int32 cast = RNE), phase = f - 2pi*k, ACT Sin.
            with tc.tile_pool(name="p3", bufs=4) as p3, \
                 tc.tile_pool(name="p3tab", bufs=3) as p3tab, \
                 tc.tile_pool(name="p3tb", bufs=10) as p3tb, \
                 tc.tile_pool(name="p3ps", bufs=4, space="PSUM") as p3ps, \
                 tc.tile_pool(name="p3psf", bufs=2, space="PSUM") as p3psf, \
                 tc.tile_pool(name="p3pss", bufs=2, space="PSUM") as p3pss:
                wk_sb = load_w("Wk")
                TWO_PI = float(2.0 * np.pi)

                def make_table(t, c, invp_sb, idxf_sb):
                    sl = slice(c * 512, (c + 1) * 512)
                    fps = p3psf.tile([128, 512], F32, tag="fps")
                    nc.tensor.matmul(fps, lhsT=invp_sb[:, t * 128:(t + 1) * 128],
                                     rhs=idxf_sb[:, sl], start=True, stop=True)
                    ki = p3tab.tile([128, 512], mybir.dt.int32, tag="ki")
                    # HW f32->i32 cast is round-to-nearest-even == round(f/2pi).
                    # (CoreSim truncates here instead; sim tables differ, HW is right.)
                    nc.vector.tensor_scalar(out=ki, in0=fps, scalar1=1.0 / TWO_PI,
                                            scalar2=None, op0=mybir.AluOpType.mult)
                    kf = p3tab.tile([128, 512], F32, tag="kf")
                    nc.vector.tensor_copy(kf, ki)
                    ph = p3tab.tile([128, 512], F32, tag="ph")
                    nc.vector.scalar_tensor_tensor(out=ph, in0=kf, scalar=-TWO_PI,
                                                   in1=fps, op0=mybir.AluOpType.mult,
                                                   op1=mybir.AluOpType.add)
                    tb = p3tb.tile([128, 512], BF16, tag="tb")
                    nc.scalar.activation(tb, ph, AF.Sin)
                    return tb

                def proj_tile(t, w_sb, bias_sb):
                    qs = p3.tile([128, S], BF16, tag="qs")
                    for c in range(2):
                        ps = p3ps.tile([128, 512], F32, tag="mm")
                        for kt in range(NT):
                            nc.tensor.matmul(
                                ps,
                                lhsT=w_sb[:, kt, t * 128:(t + 1) * 128],
                                rhs=xT[:, kt, c * 512:(c + 1) * 512],
                                start=(kt == 0), stop=(kt == NT - 1))
                        nc.scalar.activation(qs[:, c * 512:(c + 1) * 512], ps,
                                             AF.Identity, bias=bias_sb[:, t:t + 1])
                    return qs

                def rotate(t, qs, cos_c, sin_c, dst):
                    for c in range(2):
                        sl = slice(c * 512, (c + 1) * 512)
                        sh = p3pss.tile([128, 512], F32, tag="shuf")
                        nc.tensor.matmul(sh, lhsT=pmat, rhs=qs[:, sl], start=True, stop=True)
                        t1 = p3.tile([128, 512], BF16, tag="t1")
                        nc.vector.tensor_mul(t1, qs[:, sl], cos_c[c])
                        t2 = p3.tile([128, 512], BF16, tag="t2")
                        nc.vector.tensor_mul(t2, sh, sin_c[c])
                        nc.vector.tensor_add(dst[:, t, sl], t1, t2)

                for t in range(NT):
                    qs = proj_tile(t, wq_sb, bq_sb)
                    ks = proj_tile(t, wk_sb, bk_sb)
                    cos_q = [make_table(t, c, invc_sb, idxfq_sb) for c in range(2)]
                    sin_q = [make_table(t, c, invs_sb, idxfq_sb) for c in range(2)]
                    rotate(t, qs, cos_q, sin_q, qr)
                    if share_qk:
                        cos_k, sin_k = cos_q, sin_q
                    else:
                        cos_k = [make_table(t, c, invc_sb, idxfk_sb) for c in range(2)]
                        sin_k = [make_table(t, c, invs_sb, idxfk_sb) for c in range(2)]
                    rotate(t, ks, cos_k, sin_k, kr)

            if phase_limit < 4:
                nc.sync.dma_start(out=out_d.ap(), in_=x_d.ap())
                continue
            # =========== P4: attention per head ==============================
            if EXP.get("skip_p4"):
                nc.vector.tensor_copy(ctxT[:, 0, 0:512], xT[:, 0, 0:512])
            el_guard = EXP.get("skip_p4")
            if not EXP.get("skip_p4"):
             with tc.tile_pool(name="p4", bufs=2) as p4, \
                  tc.tile_pool(name="p4s", bufs=1) as p4s, \
                  tc.tile_pool(name="p4ps", bufs=2, space="PSUM") as p4ps, \
                  tc.tile_pool(name="p4psd", bufs=2, space="PSUM") as p4psd, \
                  tc.tile_pool(name="p4psc", bufs=2, space="PSUM") as p4psc:
                 den_hs = []
                 for h in range(H):
                     expT = p4.tile([128, NT, S], BF16, tag="expT")
                     den_h = p4.tile([1, S], F32, tag="den_h")
                     den_hs.append(den_h)
                     for jt in range(NT):
                         i0 = jt * 128
                         # i-regions of this j-tile, split at the 512 boundary
                         regs = []
                         if i0 < 512:
                             regs.append((i0, 512))
                         regs.append((max(i0, 512), 1024))
                         for (a, b) in regs:
                             ps = p4ps.tile([128, 512], F32, tag="sc")
                             nc.tensor.matmul(
                                 ps[:, 0:b - a],
                                 lhsT=kr[:, h, i0:i0 + 128],
                                 rhs=qr[:, h, a:b], start=True, stop=True)
                             nc.scalar.activation(expT[:, jt, a:b], ps[:, 0:b - a],
                                                  AF.Exp, scale=SCALE)
                         # causal mask on the diagonal 128x128 block
                         nc.gpsimd.affine_select(
                             out=expT[:, jt, i0:i0 + 128], in_=expT[:, jt, i0:i0 + 128],
                             pattern=[[1, 128]], compare_op=mybir.AluOpType.is_ge,
                             fill=0.0, base=0, channel_multiplier=-1)
                     # denominators + AV accumulation per 512-chunk
                     for c in range(2):
                         cs, ce = c * 512, (c + 1) * 512
                         jts = [jt for jt in range(NT) if jt * 128 < ce]
                         dps = p4psd.tile([1, 512], F32, tag="den")
                         for n, jt in enumerate(jts):
                             a = max(jt * 128, cs)
                             nc.tensor.matmul(
                                 dps[:, a - cs:512], lhsT=ones_den,
                                 rhs=expT[:, jt, a:ce],
                                 start=(n == 0), stop=(n == len(jts) - 1))
                         nc.scalar.copy(den_h[0:1, cs:ce], dps)
                         cps = p4psc.tile([128, 512], F32, tag="ctx")
                         for n, jt in enumerate(jts):
                             a = max(jt * 128, cs)
                             nc.tensor.matmul(
                                 cps[:, a - cs:512],
                                 lhsT=vsb[:, jt, h * 128:(h + 1) * 128],
                                 rhs=expT[:, jt, a:ce],
                                 start=(n == 0), stop=(n == len(jts) - 1))
                         nc.vector.tensor_copy(ctxT[:, h, cs:ce], cps)
                 # normalize: ctxT = ctxU * (1/den) broadcast along partitions
                 for h in range(H):
                     rden_h = p4.tile([1, S], F32, tag="rden_h")
                     rscr_h = p4.tile([1, S], F32, tag="rscr_h")
                     nc.vector.reciprocal_approx_accurate(
                         out=rden_h, in_=den_hs[h], scratch=rscr_h)
                     for c in range(2):
                         cs, ce = c * 512, (c + 1) * 512
                         bps = p4psc.tile([128, 512], F32, tag="rdenB")
                         nc.tensor.matmul(bps, lhsT=ones1_f,
                                          rhs=rden_h[0:1, cs:ce],
                                          start=True, stop=True)
                         nc.vector.tensor_mul(ctxT[:, h, cs:ce], ctxT[:, h, cs:ce], bps)

             if phase_limit < 5:
                 nc.sync.dma_start(out=out_d.ap(), in_=x_d.ap())
                 continue
            # =========== P5: y = relu(ctx Wo + bo)  (feature-major) ==========
            mid_ctx.__exit__(None, None, None)
            res2 = top.enter_context(tc.tile_pool(name="res2", bufs=1))
            yT = res2.tile([128, NT, S], BF16, tag="yT")     # relu(ctx Wo + bo)^T
            rx = res2.tile([128, NT, S], BF16, tag="rx")     # (r*x)^T
            with tc.tile_pool(name="p5ps", bufs=EXP.get("p5ps", 4), space="PSUM") as p5ps:
                wo_sb = load_w("Wo")
                wxr_sb = load_w("Wxr")
                wyr_sb = load_w("Wyr")
                for t in range(NT):
                    for c in range(2):
                        ps = p5ps.tile([128, 512], F32, tag="mm")
                        rhs_src = xT if EXP.get("y_rhs_xt") else ctxT
                        for kt in range(NT):
                            nc.tensor.matmul(
                                ps, lhsT=wo_sb[:, kt, t * 128:(t + 1) * 128],
                                rhs=rhs_src[:, kt, c * 512:(c + 1) * 512],
                                start=(kt == 0), stop=(kt == NT - 1))
                        if EXP.get("y_act"):
                            nc.scalar.activation(yT[:, t, c * 512:(c + 1) * 512], ps,
                                                 AF.Relu, bias=bo_sb[:, t:t + 1])
                        else:
                            nc.vector.tensor_scalar(out=yT[:, t, c * 512:(c + 1) * 512],
                                                    in0=ps, scalar1=bo_sb[:, t:t + 1],
                                                    scalar2=0.0, op0=mybir.AluOpType.add,
                                                    op1=mybir.AluOpType.max)

                # ======= P6: r = sigmoid(x Wxr + y Wyr);  rx = r * xT ========
                with tc.tile_pool(name="p6", bufs=3) as p6:
                  if phase_limit >= 6:
                      for t in range(NT):
                          for c in range(2):
                              sl = slice(c * 512, (c + 1) * 512)
                              ps = p5ps.tile([128, 512], F32, tag="mm")
                              if EXP.get("r_seq"):
                                  # timing-only: seq-major operand order (wrong math)
                                  for kt in range(NT):
                                      nc.tensor.matmul(
                                          ps, lhsT=xT[:, kt, t * 128:(t + 1) * 128],
                                          rhs=wxr_sb[:, kt, sl], start=(kt == 0), stop=False)
                                  for kt in range(NT):
                                      nc.tensor.matmul(
                                          ps, lhsT=yT[:, kt, t * 128:(t + 1) * 128],
                                          rhs=wyr_sb[:, kt, sl], start=False, stop=(kt == NT - 1))
                              else:
                                  for kt in range(NT):
                                      nc.tensor.matmul(
                                          ps, lhsT=wxr_sb[:, kt, t * 128:(t + 1) * 128],
                                          rhs=xT[:, kt, sl], start=(kt == 0), stop=False)
                                  for kt in range(NT):
                                      nc.tensor.matmul(
                                          ps, lhsT=wyr_sb[:, kt, t * 128:(t + 1) * 128],
                                          rhs=yT[:, kt, sl], start=False, stop=(kt == NT - 1))
                              rt = p6.tile([128, 512], BF16, tag="rt")
                              nc.scalar.activation(rt, ps, AF.Sigmoid)
                              nc.vector.tensor_mul(rx[:, t, sl], rt, xT[:, t, sl])

            if phase_limit < 7:
                nc.sync.dma_start(out=out_d.ap(), in_=x_d.ap())
                continue
            # =========== P7: z/h + gated combine (seq-major, single pass) ====
            # 4 weight matrices live at once: Wxz/Wyz in wpool, Wxg/Wyg in a
            # dedicated pool (avoids the 2-slot deadlock without z_all staging).
            with tc.tile_pool(name="p7", bufs=2) as p7, \
                 tc.tile_pool(name="w7", bufs=2) as w7, \
                 tc.tile_pool(name="p7ps", bufs=4, space="PSUM") as p7ps:
                wxz_sb = load_w("Wxz")
                wyz_sb = load_w("Wyz")
                wxg_sb = w7.tile([128, NT, E], BF16, tag="W7")
                nc.sync.dma_start(out=wxg_sb, in_=w_d["Wxg"].ap().rearrange("p (t e) -> p t e", t=NT))
                wyg_sb = w7.tile([128, NT, E], BF16, tag="W7")
                nc.sync.dma_start(out=wyg_sb, in_=w_d["Wyg"].ap().rearrange("p (t e) -> p t e", t=NT))
                for st in range(NT):
                    ss = slice(st * 128, (st + 1) * 128)
                    xf = p7.tile([128, E], F32, tag="xf")
                    nc.sync.dma_start(out=xf, in_=x_d.ap()[ss, :])
                    ot = p7.tile([128, E], F32, tag="ot")
                    for c in range(2):
                        sl = slice(c * 512, (c + 1) * 512)
                        zps = p7ps.tile([128, 512], F32, tag="zps")
                        for kt in range(NT):
                            nc.tensor.matmul(zps, lhsT=xT[:, kt, ss],
                                             rhs=wxz_sb[:, kt, sl],
                                             start=(kt == 0), stop=False)
                        for kt in range(NT):
                            nc.tensor.matmul(zps, lhsT=yT[:, kt, ss],
                                             rhs=wyz_sb[:, kt, sl],
                                             start=False, stop=False)
                        nc.tensor.matmul(zps, lhsT=ones1_b, rhs=bxz_row[:, sl],
                                         start=False, stop=True)
                        zt = p7.tile([128, 512], F32, tag="zt")
                        nc.scalar.activation(zt, zps, AF.Sigmoid)
                        hps = p7ps.tile([128, 512], F32, tag="hps")
                        for kt in range(NT):
                            nc.tensor.matmul(hps, lhsT=rx[:, kt, ss],
                                             rhs=wxg_sb[:, kt, sl],
                                             start=(kt == 0), stop=False)
                        for kt in range(NT):
                            nc.tensor.matmul(hps, lhsT=yT[:, kt, ss],
                                             rhs=wyg_sb[:, kt, sl],
                                             start=False, stop=(kt == NT - 1))
                        ht = p7.tile([128, 512], F32, tag="ht")
                        nc.scalar.activation(ht, hps, AF.Tanh)
                        dt = p7.tile([128, 512], F32, tag="dt")
                        nc.vector.tensor_sub(dt, ht, xf[:, sl])
                        zd = p7.tile([128, 512], F32, tag="zd")
                        nc.vector.tensor_mul(zd, zt, dt)
                        nc.vector.tensor_add(ot[:, sl], xf[:, sl], zd)
                    nc.sync.dma_start(out=out_d.ap()[ss, :], in_=ot)

    nc.compile()
    return nc


def _pack_w(w):
    return np.ascontiguousarray(
        w.astype(NPBF16).reshape(NT, 128, E).transpose(1, 0, 2).reshape(128, NT * E))


def _pack_bias_fm(b):
    return np.ascontiguousarray(b.astype(np.float32).reshape(NT, 128).T)


def _wrap_idx(idx):
    return np.ascontiguousarray(
        np.tile(idx.astype(np.int16).reshape(S // 16, 16).T, (8, 1)))


_INVP = None


def _invp():
    global _INVP
    if _INVP is None:
        inv = (1.0 / (10000.0 ** (np.arange(0, E, 2, dtype=np.float32) / np.float32(E))))
        inv_exp = np.repeat(inv.astype(np.float32), 2)  # pair-expanded [E]
        invp_sin = np.zeros((2, NT * 128), np.float32)
        invp_sin[0] = inv_exp
        invp_cos = np.zeros((2, NT * 128), np.float32)
        invp_cos[0] = inv_exp
        invp_cos[1] = np.float32(np.pi / 2)
        _INVP = (invp_sin, invp_cos)
    return _INVP


def _idxf(idx):
    m = np.ones((2, S), np.float32)
    m[0] = idx.astype(np.float32)
    return m


def _pmat():
    pm = np.zeros((128, 128), dtype=NPBF16)
    for i in range(64):
        pm[2 * i + 1, 2 * i] = -1.0
        pm[2 * i, 2 * i + 1] = 1.0
    return pm


def make_in_maps(inputs, share_qk):
    x = np.asarray(inputs["x"], dtype=np.float32)
    qi = np.asarray(inputs["query_index"])
    ki = np.asarray(inputs["key_index"])
    invp_sin, invp_cos = _invp()
    common = {
        "bq": _pack_bias_fm(np.asarray(inputs["bq"])),
        "bk": _pack_bias_fm(np.asarray(inputs["bk"])),
        "bo": _pack_bias_fm(np.asarray(inputs["bo"])),
        "bv_row": np.asarray(inputs["bv"]).astype(NPBF16).reshape(1, E),
        "bxz_row": np.asarray(inputs["bxz"]).astype(NPBF16).reshape(1, E),
        "invp_sin": invp_sin,
        "invp_cos": invp_cos,
        "ident": np.eye(128, dtype=NPBF16),
        "pmat": _pmat(),
    }
    for nm in ("Wq", "Wk", "Wv", "Wo", "Wxr", "Wyr", "Wxz", "Wyz", "Wxg", "Wyg"):
        common[nm] = _pack_w(np.asarray(inputs[nm]))
    in_maps = []
    for b in range(B):
        m = dict(common)
        m["x"] = np.ascontiguousarray(x[b])
        m["idxf_q"] = _idxf(qi[b])
        if not share_qk:
            m["idxf_k"] = _idxf(ki[b])
        in_maps.append(m)
    return in_maps


def kernel(**inputs):
    qi = np.asarray(inputs["query_index"])
    ki = np.asarray(inputs["key_index"])
    share_qk = bool(np.array_equal(qi, ki))

    key = ("k", share_qk)
    if key not in _COMPILED:
        _COMPILED[key] = _build(share_qk)
    nc = _COMPILED[key]

    in_maps = make_in_maps(inputs, share_qk)
    global _dbg_in_maps
    _dbg_in_maps = in_maps
    res = bass_utils.run_bass_kernel_spmd(nc, in_maps, core_ids=list(range(NC)))
    out = np.stack([res.results[b]["out"] for b in range(B)]).astype(np.float32)
    return out



# revision 14
# speedup vs baseline: 1.6481x; 1.6481x over previous
# Trainium2 Bass kernel for nn_EpisodeMultiheadAttentionBlock.
# B=8, S=1024, E=1024, H=8 heads, HD=128. Data-parallel over batch: core b
# computes batch element b. Self-contained: only needs /opt/trn_rl_repo on path.
#
# v2: fp8(e4m3) DoubleRow matmuls for QKV/Wo/y-side gates and attention AV/den
# (weights pre-scaled by powers of 2, dequantized via activation scales);
# host-precomputed x^T and RoPE cos/sin tables (no on-device transposes or
# table generation); bf16 everywhere precision demands it (scores, x-side
# gate matmuls, rope rotation); final gated combine in f32.
import sys
import numpy as np

sys.path.insert(0, "/opt/trn_rl_repo")

import ml_dtypes  # noqa: E402
import concourse.bass as bass  # noqa: E402
import concourse.mybir as mybir  # noqa: E402
import concourse.tile as tile  # noqa: E402
from concourse import bacc  # noqa: E402
from concourse import bass_utils  # noqa: E402

B, S, E, H = 8, 1024, 1024, 8
HD = E // H  # 128
NT = E // 128  # 8 e-tiles / s-tiles
NP = NT // 2  # 4 DoubleRow k-tile pairs
NC = 8  # cores
BF16 = mybir.dt.bfloat16
F32 = mybir.dt.float32
FP8 = mybir.dt.float8e4
AF = mybir.ActivationFunctionType
DR = mybir.MatmulPerfMode.DoubleRow
NPBF16 = ml_dtypes.bfloat16
NPFP8 = ml_dtypes.float8_e4m3

WS = 32.0  # weight pre-scale for fp8/bf16 weights
YS = 8.0   # y stored as 8*y in fp8
CS = 16.0  # ctx stored as 16*ctx in fp8

_COMPILED = {}


def _build(share_qk: bool):
    nc = bacc.Bacc("TRN2", target_bir_lowering=False, debug=False, num_devices=NC)

    # ---- DRAM tensors -------------------------------------------------------
    x_d = nc.dram_tensor("x", [S, E], F32, kind="ExternalInput")
    xtb_d = nc.dram_tensor("xtb", [128, NT * S], BF16, kind="ExternalInput")
    xt8_d = nc.dram_tensor("xt8", [128, NT * S], FP8, kind="ExternalInput")
    w8_d = {
        nm: nc.dram_tensor(nm, [128, NT * E], FP8, kind="ExternalInput")
        for nm in ("Wq", "Wk", "Wv", "Wo", "Wyr", "Wyz", "Wyg")
    }
    wb_d = {
        nm: nc.dram_tensor(nm, [128, NT * E], BF16, kind="ExternalInput")
        for nm in ("Wxr", "Wxz", "Wxg")
    }
    bq_d = nc.dram_tensor("bq", [128, NT], F32, kind="ExternalInput")
    bk_d = nc.dram_tensor("bk", [128, NT], F32, kind="ExternalInput")
    bo8_d = nc.dram_tensor("bo8", [128, NT], F32, kind="ExternalInput")
    bv_row_d = nc.dram_tensor("bv_row", [1, E], BF16, kind="ExternalInput")   # 32*bv
    bxz_row_d = nc.dram_tensor("bxz_row", [1, E], BF16, kind="ExternalInput")  # 32*bxz
    cosq_d = nc.dram_tensor("cosq", [128, NT * S], BF16, kind="ExternalInput")
    sinq_d = nc.dram_tensor("sinq", [128, NT * S], BF16, kind="ExternalInput")
    if not share_qk:
        cosk_d = nc.dram_tensor("cosk", [128, NT * S], BF16, kind="ExternalInput")
        sink_d = nc.dram_tensor("sink", [128, NT * S], BF16, kind="ExternalInput")
    pmat_d = nc.dram_tensor("pmat", [128, 128], BF16, kind="ExternalInput")
    ones8_d = nc.dram_tensor("ones8", [128, 256], FP8, kind="ExternalInput")
    out_d = nc.dram_tensor("out", [S, E], F32, kind="ExternalOutput")

    SCALE = 1.0 / float(np.sqrt(HD))

    def r3(ap):
        return ap.rearrange("p (t s) -> p t s", t=NT)

    with tile.TileContext(nc) as tc:
      from contextlib import ExitStack

      with ExitStack() as top:
        # ---------------- resident tiles -----------------------------------
        res = top.enter_context(tc.tile_pool(name="res", bufs=1))
        xTb = res.tile([128, NT, S], BF16, tag="xTb")
        nc.sync.dma_start(out=xTb, in_=r3(xtb_d.ap()))
        xT8 = res.tile([128, NT, S], FP8, tag="xT8")
        nc.sync.dma_start(out=xT8, in_=r3(xt8_d.ap()))

        consts = top.enter_context(tc.tile_pool(name="consts", bufs=1))
        pmat = consts.tile([128, 128], BF16, tag="pmat")
        nc.sync.dma_start(out=pmat, in_=pmat_d.ap())
        bq_sb = consts.tile([128, NT], F32, tag="bq")
        nc.sync.dma_start(out=bq_sb, in_=bq_d.ap())
        bk_sb = consts.tile([128, NT], F32, tag="bk")
        nc.sync.dma_start(out=bk_sb, in_=bk_d.ap())
        bo8_sb = consts.tile([128, NT], F32, tag="bo8")
        nc.sync.dma_start(out=bo8_sb, in_=bo8_d.ap())
        bv_row = consts.tile([1, E], BF16, tag="bv_row")
        nc.sync.dma_start(out=bv_row, in_=bv_row_d.ap())
        bxz_row = consts.tile([1, E], BF16, tag="bxz_row")
        nc.sync.dma_start(out=bxz_row, in_=bxz_row_d.ap())
        ones8 = consts.tile([128, 2, 128], FP8, tag="ones8")  # value 1/CS
        nc.sync.dma_start(out=ones8, in_=ones8_d.ap().rearrange("p (a b) -> p a b", a=2))
        ones1_b = consts.tile([1, 128], BF16, tag="ones1_b")
        nc.vector.memset(ones1_b, 1.0)

        # weight streaming pool (slots sized [128, NT, E]; fp8 and bf16 mix)
        wp8 = top.enter_context(tc.tile_pool(name="wp8", bufs=3))
        wpb = top.enter_context(tc.tile_pool(name="wpb", bufs=2))

        def load_w8(nm):
            t = wp8.tile([128, NT, E], FP8, tag="W8")
            nc.sync.dma_start(out=t, in_=w8_d[nm].ap().rearrange("p (t e) -> p t e", t=NT))
            return t

        def load_wb(nm):
            t = wpb.tile([128, NT, E], BF16, tag="Wb")
            nc.sync.dma_start(out=t, in_=wb_d[nm].ap().rearrange("p (t e) -> p t e", t=NT))
            return t

        # mid tiles live through P4 only
        mid_ctx = tc.tile_pool(name="mid", bufs=1)
        mid = mid_ctx.__enter__()
        vsb8 = mid.tile([128, NT, E], FP8, tag="vsb8")   # v/32 in fp8   [s, e]
        qr = mid.tile([128, NT, S], BF16, tag="qr")      # rope(q)^T
        kr = mid.tile([128, NT, S], BF16, tag="kr")      # rope(k)^T
        cosq = mid.tile([128, NT, S], BF16, tag="cosq")
        nc.sync.dma_start(out=cosq, in_=r3(cosq_d.ap()))
        sinq = mid.tile([128, NT, S], BF16, tag="sinq")
        nc.sync.dma_start(out=sinq, in_=r3(sinq_d.ap()))
        if share_qk:
            cosk, sink = cosq, sinq
        else:
            cosk = mid.tile([128, NT, S], BF16, tag="cosk")
            nc.sync.dma_start(out=cosk, in_=r3(cosk_d.ap()))
            sink = mid.tile([128, NT, S], BF16, tag="sink")
            nc.sync.dma_start(out=sink, in_=r3(sink_d.ap()))

        # =========== P2: v/32 = (xT8^T @ Wv8)/32 + bv  (seq-major) ==========
        wv_sb = load_w8("Wv")
        wq_sb = load_w8("Wq")
        with tc.tile_pool(name="p2ps", bufs=4, space="PSUM") as p2ps:
            for st in range(NT):
                ss = slice(st * 128, (st + 1) * 128)
                for c in range(2):
                    sl = slice(c * 512, (c + 1) * 512)
                    ps = p2ps.tile([128, 512], F32, tag="mm")
                    for kp in range(NP):
                        nc.tensor.matmul(
                            ps, lhsT=xT8[:, 2 * kp:2 * kp + 2, ss],
                            rhs=wv_sb[:, 2 * kp:2 * kp + 2, sl],
                            start=(kp == 0), stop=False, perf_mode=DR)
                    nc.tensor.matmul(ps, lhsT=ones1_b, rhs=bv_row[:, sl],
                                     start=False, stop=True)
                    # vsb8 = psum/32 in fp8
                    nc.vector.tensor_scalar(out=vsb8[:, st, sl], in0=ps,
                                            scalar1=1.0 / WS, scalar2=None,
                                            op0=mybir.AluOpType.mult)

        # =========== P3: q/k proj (fp8 DR) + RoPE (bf16) =====================
        with tc.tile_pool(name="p3", bufs=4) as p3, \
             tc.tile_pool(name="p3ps", bufs=4, space="PSUM") as p3ps, \
             tc.tile_pool(name="p3pss", bufs=2, space="PSUM") as p3pss:
            wk_sb = load_w8("Wk")

            def proj_tile(t, w_sb, bias_sb):
                qs = p3.tile([128, S], BF16, tag="qs")
                for c in range(2):
                    sl = slice(c * 512, (c + 1) * 512)
                    ps = p3ps.tile([128, 512], F32, tag="mm")
                    for kp in range(NP):
                        nc.tensor.matmul(
                            ps, lhsT=w_sb[:, 2 * kp:2 * kp + 2, t * 128:(t + 1) * 128],
                            rhs=xT8[:, 2 * kp:2 * kp + 2, sl],
                            start=(kp == 0), stop=(kp == NP - 1), perf_mode=DR)
                    nc.scalar.activation(qs[:, sl], ps, AF.Identity,
                                         bias=bias_sb[:, t:t + 1], scale=1.0 / WS)
                return qs

            def rotate(t, qs, cos_t, sin_t, dst):
                for c in range(2):
                    sl = slice(c * 512, (c + 1) * 512)
                    sh = p3pss.tile([128, 512], F32, tag="shuf")
                    nc.tensor.matmul(sh, lhsT=pmat, rhs=qs[:, sl], start=True, stop=True)
                    t1 = p3.tile([128, 512], BF16, tag="t1")
                    nc.vector.tensor_mul(t1, qs[:, sl], cos_t[:, t, sl])
                    t2 = p3.tile([128, 512], BF16, tag="t2")
                    nc.vector.tensor_mul(t2, sh, sin_t[:, t, sl])
                    nc.vector.tensor_add(dst[:, t, sl], t1, t2)

            for t in range(NT):
                qs = proj_tile(t, wq_sb, bq_sb)
                rotate(t, qs, cosq, sinq, qr)
                ks = proj_tile(t, wk_sb, bk_sb)
                rotate(t, ks, cosk, sink, kr)

        # =========== P4: attention per head ==================================
        with tc.tile_pool(name="p4", bufs=2) as p4, \
             tc.tile_pool(name="p4ps", bufs=2, space="PSUM") as p4ps, \
             tc.tile_pool(name="p4psd", bufs=2, space="PSUM") as p4psd, \
             tc.tile_pool(name="p4psc", bufs=2, space="PSUM") as p4psc:
            ctx8 = res.tile([128, NT, S], FP8, tag="ctx8")  # 16*ctx in fp8
            for h in range(H):
                expT = p4.tile([128, NT, S], FP8, tag="expT")
                for jt in range(NT):
                    i0 = jt * 128
                    regs = []
                    if i0 < 512:
                        regs.append((i0, 512))
                    regs.append((max(i0, 512), 1024))
                    for (a, b) in regs:
                        ps = p4ps.tile([128, 512], F32, tag="sc")
                        nc.tensor.matmul(
                            ps[:, 0:b - a],
                            lhsT=kr[:, h, i0:i0 + 128],
                            rhs=qr[:, h, a:b], start=True, stop=True)
                        nc.scalar.activation(expT[:, jt, a:b], ps[:, 0:b - a],
                                             AF.Exp, scale=SCALE)
                    # causal mask on the diagonal 128x128 block
                    nc.gpsimd.affine_select(
                        out=expT[:, jt, i0:i0 + 128], in_=expT[:, jt, i0:i0 + 128],
                        pattern=[[1, 128]], compare_op=mybir.AluOpType.is_ge,
                        fill=0.0, base=0, channel_multiplier=-1)
                    # zero the strip a DoubleRow pair-partner reads above the
                    # diagonal: odd tiles cover queries [i0-128, i0) as zeros
                    if jt % 2 == 1:
                        nc.vector.memset(expT[:, jt, i0 - 128:i0], 0.0)
                # denominators + AV accumulation per 512-chunk (DR over jt pairs)
                # den broadcast to all 128 partitions via ones(1/CS) lhsT, so
                # reciprocal directly yields the CS/den normalizer per query.
                for c in range(2):
                    cs, ce = c * 512, (c + 1) * 512
                    jps = [jp for jp in range(NP) if jp * 256 < ce]
                    dps = p4psd.tile([128, 512], F32, tag="den")
                    for n, jp in enumerate(jps):
                        a = max(jp * 256, cs)
                        nc.tensor.matmul(
                            dps[:, a - cs:512], lhsT=ones8,
                            rhs=expT[:, 2 * jp:2 * jp + 2, a:ce],
                            start=(n == 0), stop=(n == len(jps) - 1), perf_mode=DR)
                    rf = p4.tile([128, 512], F32, tag="rf")
                    nc.vector.reciprocal_approx_fast(out=rf, in_=dps)
                    cps = p4psc.tile([128, 512], F32, tag="ctx")
                    for n, jp in enumerate(jps):
                        a = max(jp * 256, cs)
                        nc.tensor.matmul(
                            cps[:, a - cs:512],
                            lhsT=vsb8[:, 2 * jp:2 * jp + 2, h * 128:(h + 1) * 128],
                            rhs=expT[:, 2 * jp:2 * jp + 2, a:ce],
                            start=(n == 0), stop=(n == len(jps) - 1), perf_mode=DR)
                    nc.vector.tensor_mul(ctx8[:, h, cs:ce], cps, rf)

        # =========== P5: y8 = 8*relu(ctx Wo + bo)  (feature-major) ==========
        mid_ctx.__exit__(None, None, None)
        res2 = top.enter_context(tc.tile_pool(name="res2", bufs=1))
        yT8 = res2.tile([128, NT, S], FP8, tag="yT8")    # 8*y in fp8
        rx = res2.tile([128, NT, S], BF16, tag="rx")     # (r*x)^T bf16
        with tc.tile_pool(name="p5ps", bufs=4, space="PSUM") as p5ps:
            wo_sb = load_w8("Wo")
            wxr_sb = load_wb("Wxr")
            wyr_sb = load_w8("Wyr")
            for t in range(NT):
                for c in range(2):
                    sl = slice(c * 512, (c + 1) * 512)
                    ps = p5ps.tile([128, 512], F32, tag="mm")
                    for kp in range(NP):
                        nc.tensor.matmul(
                            ps, lhsT=wo_sb[:, 2 * kp:2 * kp + 2, t * 128:(t + 1) * 128],
                            rhs=ctx8[:, 2 * kp:2 * kp + 2, sl],
                            start=(kp == 0), stop=(kp == NP - 1), perf_mode=DR)
                    # psum = WS*CS*(ctx@Wo); y8 = relu(psum*YS/(WS*CS) + YS*bo)
                    nc.scalar.activation(yT8[:, t, sl], ps, AF.Relu,
                                         bias=bo8_sb[:, t:t + 1],
                                         scale=YS / (WS * CS))

            # ===== P6: r = sigmoid(x Wxr + y Wyr); rx = r * xT (bf16) ========
            with tc.tile_pool(name="p6", bufs=3) as p6:
                for t in range(NT):
                    for c in range(2):
                        sl = slice(c * 512, (c + 1) * 512)
                        ps = p5ps.tile([128, 512], F32, tag="mm")
                        for kt in range(NT):
                            nc.tensor.matmul(
                                ps, lhsT=wxr_sb[:, kt, t * 128:(t + 1) * 128],
                                rhs=xTb[:, kt, sl], start=(kt == 0), stop=False)
                        for kp in range(NP):
                            nc.tensor.matmul(
                                ps, lhsT=wyr_sb[:, 2 * kp:2 * kp + 2, t * 128:(t + 1) * 128],
                                rhs=yT8[:, 2 * kp:2 * kp + 2, sl],
                                start=False, stop=(kp == NP - 1), perf_mode=DR)
                        rt = p6.tile([128, 512], BF16, tag="rt")
                        nc.scalar.activation(rt, ps, AF.Sigmoid, scale=1.0 / WS)
                        nc.vector.tensor_mul(rx[:, t, sl], rt, xTb[:, t, sl])

        # =========== P7: z/h + gated combine (seq-major, single pass) ========
        with tc.tile_pool(name="p7", bufs=2) as p7, \
             tc.tile_pool(name="w7", bufs=2) as w7, \
             tc.tile_pool(name="p7ps", bufs=4, space="PSUM") as p7ps:
            wxz_sb = load_wb("Wxz")
            wyz_sb = load_w8("Wyz")
            wxg_sb = w7.tile([128, NT, E], BF16, tag="W7")
            nc.sync.dma_start(out=wxg_sb, in_=wb_d["Wxg"].ap().rearrange("p (t e) -> p t e", t=NT))
            wyg_sb = w7.tile([128, NT, E], FP8, tag="W7f8")
            nc.sync.dma_start(out=wyg_sb, in_=w8_d["Wyg"].ap().rearrange("p (t e) -> p t e", t=NT))
            for st in range(NT):
                ss = slice(st * 128, (st + 1) * 128)
                xf = p7.tile([128, E], F32, tag="xf")
                nc.sync.dma_start(out=xf, in_=x_d.ap()[ss, :])
                ot = p7.tile([128, E], F32, tag="ot")
                for c in range(2):
                    sl = slice(c * 512, (c + 1) * 512)
                    zps = p7ps.tile([128, 512], F32, tag="zps")
                    for kt in range(NT):
                        nc.tensor.matmul(zps, lhsT=xTb[:, kt, ss],
                                         rhs=wxz_sb[:, kt, sl],
                                         start=(kt == 0), stop=False)
                    for kp in range(NP):
                        nc.tensor.matmul(zps, lhsT=yT8[:, 2 * kp:2 * kp + 2, ss],
                                         rhs=wyz_sb[:, 2 * kp:2 * kp + 2, sl],
                                         start=False, stop=False, perf_mode=DR)
                    nc.tensor.matmul(zps, lhsT=ones1_b, rhs=bxz_row[:, sl],
                                     start=False, stop=True)
                    zt = p7.tile([128, 512], F32, tag="zt")
                    nc.scalar.activation(zt, zps, AF.Sigmoid, scale=1.0 / WS)
                    hps = p7ps.tile([128, 512], F32, tag="hps")
                    for kt in range(NT):
                        nc.tensor.matmul(hps, lhsT=rx[:, kt, ss],
                                         rhs=wxg_sb[:, kt, sl],
                                         start=(kt == 0), stop=False)
                    for kp in range(NP):
                        nc.tensor.matmul(hps, lhsT=yT8[:, 2 * kp:2 * kp + 2, ss],
                                         rhs=wyg_sb[:, 2 * kp:2 * kp + 2, sl],
                                         start=False, stop=(kp == NP - 1), perf_mode=DR)
                    ht = p7.tile([128, 512], F32, tag="ht")
                    nc.scalar.activation(ht, hps, AF.Tanh, scale=1.0 / WS)
                    dt = p7.tile([128, 512], F32, tag="dt")
                    nc.vector.tensor_sub(dt, ht, xf[:, sl])
                    zd = p7.tile([128, 512], F32, tag="zd")
                    nc.vector.tensor_mul(zd, zt, dt)
                    nc.vector.tensor_add(ot[:, sl], xf[:, sl], zd)
                nc.sync.dma_start(out=out_d.ap()[ss, :], in_=ot)

    nc.compile()
    return nc


# ---------------- host-side packing -----------------------------------------

def _pack_w(w, scale, npdt):
    return np.ascontiguousarray(
        (np.asarray(w, np.float32) * scale).astype(npdt)
        .reshape(NT, 128, E).transpose(1, 0, 2).reshape(128, NT * E))


def _pack_fm(m, npdt):
    # [E, S]-logical feature-major -> [128, NT*S]
    return np.ascontiguousarray(
        m.astype(npdt).reshape(NT, 128, S).transpose(1, 0, 2).reshape(128, NT * S))


def _pack_bias_fm(b, scale=1.0):
    return np.ascontiguousarray(
        (np.asarray(b, np.float32) * scale).reshape(NT, 128).T)


def _pmat():
    pm = np.zeros((128, 128), dtype=NPBF16)
    for i in range(64):
        pm[2 * i + 1, 2 * i] = -1.0
        pm[2 * i, 2 * i + 1] = 1.0
    return pm


_INV = None


def _inv_pair():
    global _INV
    if _INV is None:
        inv = 1.0 / (10000.0 ** (np.arange(0, E, 2, dtype=np.float32) / np.float32(E)))
        _INV = np.repeat(inv.astype(np.float64), 2)  # pair-expanded [E]
    return _INV


def _tables(idx):
    f = _inv_pair()[:, None] * idx.astype(np.float64)[None, :]  # [E, S]
    return (_pack_fm(np.cos(f).astype(np.float32), NPBF16),
            _pack_fm(np.sin(f).astype(np.float32), NPBF16))


def make_in_maps(inputs, share_qk):
    x = np.asarray(inputs["x"], dtype=np.float32)
    qi = np.asarray(inputs["query_index"])
    ki = np.asarray(inputs["key_index"])
    common = {
        "bq": _pack_bias_fm(np.asarray(inputs["bq"])),
        "bk": _pack_bias_fm(np.asarray(inputs["bk"])),
        "bo8": _pack_bias_fm(np.asarray(inputs["bo"]), YS),
        "bv_row": (np.asarray(inputs["bv"], np.float32) * WS).astype(NPBF16).reshape(1, E),
        "bxz_row": (np.asarray(inputs["bxz"], np.float32) * WS).astype(NPBF16).reshape(1, E),
        "pmat": _pmat(),
        "ones8": np.full((128, 256), 1.0 / CS, NPFP8),
    }
    for nm in ("Wq", "Wk", "Wv", "Wo"):
        common[nm] = _pack_w(inputs[nm], WS, NPFP8)
    for nm in ("Wyr", "Wyz", "Wyg"):
        common[nm] = _pack_w(inputs[nm], WS / YS, NPFP8)
    for nm in ("Wxr", "Wxz", "Wxg"):
        common[nm] = _pack_w(inputs[nm], WS, NPBF16)
    in_maps = []
    for b in range(B):
        m = dict(common)
        xb = np.ascontiguousarray(x[b])
        m["x"] = xb
        xt = xb.T  # [E, S]
        m["xtb"] = _pack_fm(xt, NPBF16)
        m["xt8"] = _pack_fm(xt, NPFP8)
        m["cosq"], m["sinq"] = _tables(qi[b])
        if not share_qk:
            m["cosk"], m["sink"] = _tables(ki[b])
        in_maps.append(m)
    return in_maps


def kernel(**inputs):
    qi = np.asarray(inputs["query_index"])
    ki = np.asarray(inputs["key_index"])
    share_qk = bool(np.array_equal(qi, ki))

    key = ("k", share_qk)
    if key not in _COMPILED:
        _COMPILED[key] = _build(share_qk)
    nc = _COMPILED[key]

    in_maps = make_in_maps(inputs, share_qk)
    global _dbg_in_maps
    _dbg_in_maps = in_maps
    res = bass_utils.run_bass_kernel_spmd(nc, in_maps, core_ids=list(range(NC)))
    out = np.stack([res.results[b]["out"] for b in range(B)]).astype(np.float32)
    return out


# revision 17
# speedup vs baseline: 1.7361x; 1.0534x over previous
# Trainium2 Bass kernel for nn_EpisodeMultiheadAttentionBlock.
# B=8, S=1024, E=1024, H=8 heads, HD=128. Data-parallel over batch: core b
# computes batch element b. Self-contained: only needs /opt/trn_rl_repo on path.
#
# v3: fp8(e4m3) DoubleRow matmuls for QKV/Wo/y-side gates and attention AV/den
# (weights pre-scaled by powers of 2, dequantized via activation scales);
# host-precomputed x^T and RoPE cos/sin tables; bf16 where precision demands
# it (scores, x-side gate matmuls, rope rotation). Single long-lived PSUM
# pool (no phase barriers), DMA loads ordered for prefetch, elementwise work
# spread across DVE/Act/Pool.
import sys
import numpy as np

sys.path.insert(0, "/opt/trn_rl_repo")

import ml_dtypes  # noqa: E402
import concourse.bass as bass  # noqa: E402
import concourse.mybir as mybir  # noqa: E402
import concourse.tile as tile  # noqa: E402
from concourse import bacc  # noqa: E402
from concourse import bass_utils  # noqa: E402

B, S, E, H = 8, 1024, 1024, 8
HD = E // H  # 128
NT = E // 128  # 8 e-tiles / s-tiles
NP = NT // 2  # 4 DoubleRow k-tile pairs
NC = 8  # cores
BF16 = mybir.dt.bfloat16
F32 = mybir.dt.float32
FP8 = mybir.dt.float8e4
AF = mybir.ActivationFunctionType
DR = mybir.MatmulPerfMode.DoubleRow
ALU = mybir.AluOpType
NPBF16 = ml_dtypes.bfloat16
NPFP8 = ml_dtypes.float8_e4m3

WS = 32.0  # weight pre-scale for fp8/bf16 weights
YS = 8.0   # y stored as 8*y in fp8
CS = 16.0  # ctx stored as 16*ctx in fp8

_COMPILED = {}


def _build(share_qk: bool):
    nc = bacc.Bacc("TRN2", target_bir_lowering=False, debug=False, num_devices=NC)

    # ---- DRAM tensors -------------------------------------------------------
    xb_d = nc.dram_tensor("xb", [S, E], BF16, kind="ExternalInput")
    xtb_d = nc.dram_tensor("xtb", [128, NT * S], BF16, kind="ExternalInput")
    xt8_d = nc.dram_tensor("xt8", [128, NT * S], FP8, kind="ExternalInput")
    w8_d = {
        nm: nc.dram_tensor(nm, [128, NT * E], FP8, kind="ExternalInput")
        for nm in ("Wq", "Wk", "Wv", "Wo", "Wyr", "Wyz", "Wyg")
    }
    wb_d = {
        nm: nc.dram_tensor(nm, [128, NT * E], BF16, kind="ExternalInput")
        for nm in ("Wxr", "Wxz", "Wxg")
    }
    bq_d = nc.dram_tensor("bq", [128, NT], F32, kind="ExternalInput")
    bk_d = nc.dram_tensor("bk", [128, NT], F32, kind="ExternalInput")
    bo8_d = nc.dram_tensor("bo8", [128, NT], F32, kind="ExternalInput")
    bv_row_d = nc.dram_tensor("bv_row", [1, E], BF16, kind="ExternalInput")   # 32*bv
    bxz_row_d = nc.dram_tensor("bxz_row", [1, E], BF16, kind="ExternalInput")  # 32*bxz
    cosq_d = nc.dram_tensor("cosq", [128, NT * S], BF16, kind="ExternalInput")
    sinq_d = nc.dram_tensor("sinq", [128, NT * S], BF16, kind="ExternalInput")
    if not share_qk:
        cosk_d = nc.dram_tensor("cosk", [128, NT * S], BF16, kind="ExternalInput")
        sink_d = nc.dram_tensor("sink", [128, NT * S], BF16, kind="ExternalInput")
    ones8_d = nc.dram_tensor("ones8", [128, 256], FP8, kind="ExternalInput")
    out_d = nc.dram_tensor("out", [S, E], F32, kind="ExternalOutput")

    SCALE = 1.0 / float(np.sqrt(HD))

    def r3(ap):
        return ap.rearrange("p (t s) -> p t s", t=NT)

    with tile.TileContext(nc) as tc:
      from contextlib import ExitStack

      with ExitStack() as top:
        res = top.enter_context(tc.tile_pool(name="res", bufs=1))
        consts = top.enter_context(tc.tile_pool(name="consts", bufs=1))
        wp8 = top.enter_context(tc.tile_pool(name="wp8", bufs=4))
        wpb = top.enter_context(tc.tile_pool(name="wpb", bufs=2))
        psum = top.enter_context(tc.tile_pool(name="psum", bufs=1, space="PSUM"))

        def load_w8(nm):
            t = wp8.tile([128, NT, E], FP8, tag="W8", name=f"w_{nm}")
            nc.sync.dma_start(out=t, in_=w8_d[nm].ap().rearrange("p (t e) -> p t e", t=NT))
            return t

        def load_wb(nm):
            t = wpb.tile([128, NT, E], BF16, tag="Wb", name=f"w_{nm}")
            nc.sync.dma_start(out=t, in_=wb_d[nm].ap().rearrange("p (t e) -> p t e", t=NT))
            return t

        # ---------------- loads in prefetch order ---------------------------
        xT8 = res.tile([128, NT, S], FP8, tag="xT8")
        nc.sync.dma_start(out=xT8, in_=r3(xt8_d.ap()))
        wv_sb = load_w8("Wv")
        wq_sb = load_w8("Wq")
        xTb = res.tile([128, NT, S], BF16, tag="xTb")
        nc.sync.dma_start(out=xTb, in_=r3(xtb_d.ap()))
        wk_sb = load_w8("Wk")

        bq_sb = consts.tile([128, NT], F32, tag="bq")
        nc.sync.dma_start(out=bq_sb, in_=bq_d.ap())
        bk_sb = consts.tile([128, NT], F32, tag="bk")
        nc.sync.dma_start(out=bk_sb, in_=bk_d.ap())
        bo8_sb = consts.tile([128, NT], F32, tag="bo8")
        nc.sync.dma_start(out=bo8_sb, in_=bo8_d.ap())
        bv_row = consts.tile([1, E], BF16, tag="bv_row")
        nc.sync.dma_start(out=bv_row, in_=bv_row_d.ap())
        bxz_row = consts.tile([1, E], BF16, tag="bxz_row")
        nc.sync.dma_start(out=bxz_row, in_=bxz_row_d.ap())
        ones8 = consts.tile([128, 2, 128], FP8, tag="ones8")  # value 1/CS
        nc.sync.dma_start(out=ones8, in_=ones8_d.ap().rearrange("p (a b) -> p a b", a=2))
        ones1_b = consts.tile([1, 128], BF16, tag="ones1_b")
        nc.vector.memset(ones1_b, 1.0)

        # mid tiles live through P4 only
        mid_ctx = tc.tile_pool(name="mid", bufs=1)
        mid = mid_ctx.__enter__()
        cosq = mid.tile([128, NT, S], BF16, tag="cosq")
        nc.sync.dma_start(out=cosq, in_=r3(cosq_d.ap()))
        sinq = mid.tile([128, NT, S], BF16, tag="sinq")
        nc.sync.dma_start(out=sinq, in_=r3(sinq_d.ap()))
        if share_qk:
            cosk, sink = cosq, sinq
        else:
            cosk = mid.tile([128, NT, S], BF16, tag="cosk")
            nc.sync.dma_start(out=cosk, in_=r3(cosk_d.ap()))
            sink = mid.tile([128, NT, S], BF16, tag="sink")
            nc.sync.dma_start(out=sink, in_=r3(sink_d.ap()))
        vsb8 = mid.tile([128, NT, E], FP8, tag="vsb8")   # v in fp8  [s, e]
        qr = mid.tile([128, NT, S], BF16, tag="qr")      # rope(q)^T
        kr = mid.tile([128, NT, S], BF16, tag="kr")      # rope(k)^T

        # prefetch the rest of the weights (slots free up as phases finish)
        wo_sb = load_w8("Wo")
        wxr_sb = load_wb("Wxr")
        wyr_sb = load_w8("Wyr")
        wxz_sb = load_wb("Wxz")
        wyz_sb = load_w8("Wyz")
        wxg_sb = load_wb("Wxg")
        wyg_sb = load_w8("Wyg")

        # =========== P2: v = (x @ Wv) + bv  (seq-major, fp8 out) ============
        for st in range(NT):
            ss = slice(st * 128, (st + 1) * 128)
            for c in range(2):
                sl = slice(c * 512, (c + 1) * 512)
                ps = psum.tile([128, 512], F32, tag="mm", bufs=2, name="ps_v")
                for kp in range(NP):
                    nc.tensor.matmul(
                        ps, lhsT=xT8[:, 2 * kp:2 * kp + 2, ss],
                        rhs=wv_sb[:, 2 * kp:2 * kp + 2, sl],
                        start=(kp == 0), stop=False, perf_mode=DR)
                nc.tensor.matmul(ps, lhsT=ones1_b, rhs=bv_row[:, sl],
                                 start=False, stop=True)
                nc.vector.tensor_scalar(out=vsb8[:, st, sl], in0=ps,
                                        scalar1=1.0 / WS, scalar2=None,
                                        op0=ALU.mult)

        # =========== P3: q/k proj (fp8 DR) + RoPE (bf16) =====================
        with tc.tile_pool(name="p3", bufs=4) as p3:
            def proj_tile(t, w_sb, bias_sb):
                qs = p3.tile([128, S], BF16, tag="qs")
                for c in range(2):
                    sl = slice(c * 512, (c + 1) * 512)
                    ps = psum.tile([128, 512], F32, tag="mm", bufs=2, name="ps_qk")
                    for kp in range(NP):
                        nc.tensor.matmul(
                            ps, lhsT=w_sb[:, 2 * kp:2 * kp + 2, t * 128:(t + 1) * 128],
                            rhs=xT8[:, 2 * kp:2 * kp + 2, sl],
                            start=(kp == 0), stop=(kp == NP - 1), perf_mode=DR)
                    nc.scalar.activation(qs[:, sl], ps, AF.Identity,
                                         bias=bias_sb[:, t:t + 1], scale=1.0 / WS)
                return qs

            def rotate(t, qs, cos_t, sin_t, dst):
                # dst = qs*cos + swap_pairs(qs)*sin' with the pair-swap done
                # by a negative-stride AP and the +- signs baked into sin'.
                for c in range(2):
                    sl = slice(c * 512, (c + 1) * 512)
                    t1 = p3.tile([128, 512], BF16, tag="t1")
                    nc.vector.tensor_mul(t1, qs[:, sl], cos_t[:, t, sl])
                    qsw = qs[:, sl].rearrange("p (a b) -> p a b", b=2)[:, :, ::-1]
                    t2 = p3.tile([128, 512], BF16, tag="t2")
                    nc.vector.tensor_mul(t2, qsw, sin_t[:, t, sl])
                    nc.vector.tensor_add(dst[:, t, sl], t1, t2)

            for t in range(NT):
                qs = proj_tile(t, wq_sb, bq_sb)
                rotate(t, qs, cosq, sinq, qr)
                ks = proj_tile(t, wk_sb, bk_sb)
                rotate(t, ks, cosk, sink, kr)

        # =========== P4: attention per head ==================================
        with tc.tile_pool(name="p4", bufs=2) as p4:
            ctx8 = res.tile([128, NT, S], FP8, tag="ctx8")  # 16*ctx in fp8
            for h in range(H):
                expT = p4.tile([128, NT, S], FP8, tag="expT")
                for jt in range(NT):
                    i0 = jt * 128
                    ps = psum.tile([128, 1024], F32, tag="sc", bufs=2, name="ps_sc")
                    if i0 < 512:
                        nc.tensor.matmul(
                            ps[:, i0:512],
                            lhsT=kr[:, h, i0:i0 + 128],
                            rhs=qr[:, h, i0:512], start=True, stop=True)
                        nc.tensor.matmul(
                            ps[:, 512:1024],
                            lhsT=kr[:, h, i0:i0 + 128],
                            rhs=qr[:, h, 512:1024], start=True, stop=True)
                    else:
                        nc.tensor.matmul(
                            ps[:, i0:1024],
                            lhsT=kr[:, h, i0:i0 + 128],
                            rhs=qr[:, h, i0:1024], start=True, stop=True)
                    nc.scalar.activation(expT[:, jt, i0:1024], ps[:, i0:1024],
                                         AF.Exp, scale=SCALE)
                    # causal mask on the diagonal 128x128 block
                    nc.gpsimd.affine_select(
                        out=expT[:, jt, i0:i0 + 128], in_=expT[:, jt, i0:i0 + 128],
                        pattern=[[1, 128]], compare_op=ALU.is_ge,
                        fill=0.0, base=0, channel_multiplier=-1)
                    # zero the strip the DoubleRow pair-partner reads above
                    # the diagonal (odd tiles, queries [i0-128, i0))
                    if jt % 2 == 1:
                        nc.vector.memset(expT[:, jt, i0 - 128:i0], 0.0)
                # den broadcast to all partitions via ones(1/CS) lhsT, then
                # reciprocal directly yields the CS/den normalizer per query.
                for c in range(2):
                    cs, ce = c * 512, (c + 1) * 512
                    jps = [jp for jp in range(NP) if jp * 256 < ce]
                    dps = psum.tile([128, 512], F32, tag="dc", bufs=2, name="ps_den")
                    for n, jp in enumerate(jps):
                        a = max(jp * 256, cs)
                        nc.tensor.matmul(
                            dps[:, a - cs:512], lhsT=ones8,
                            rhs=expT[:, 2 * jp:2 * jp + 2, a:ce],
                            start=(n == 0), stop=(n == len(jps) - 1), perf_mode=DR)
                    rf = p4.tile([128, 512], F32, tag="rf")
                    nc.vector.reciprocal_approx_fast(out=rf, in_=dps)
                    cps = psum.tile([128, 512], F32, tag="dc", bufs=2, name="ps_ctx")
                    for n, jp in enumerate(jps):
                        a = max(jp * 256, cs)
                        nc.tensor.matmul(
                            cps[:, a - cs:512],
                            lhsT=vsb8[:, 2 * jp:2 * jp + 2, h * 128:(h + 1) * 128],
                            rhs=expT[:, 2 * jp:2 * jp + 2, a:ce],
                            start=(n == 0), stop=(n == len(jps) - 1), perf_mode=DR)
                    nc.vector.tensor_mul(ctx8[:, h, cs:ce], cps, rf)

        # =========== P5: y8 = 8*relu(ctx Wo + bo)  (feature-major) ==========
        mid_ctx.__exit__(None, None, None)
        res2 = top.enter_context(tc.tile_pool(name="res2", bufs=1))
        yT8 = res2.tile([128, NT, S], FP8, tag="yT8")    # 8*y in fp8
        rx = res2.tile([128, NT, S], BF16, tag="rx")     # (r*x)^T bf16
        for t in range(NT):
            for c in range(2):
                sl = slice(c * 512, (c + 1) * 512)
                ps = psum.tile([128, 512], F32, tag="mm", bufs=2, name="ps_y")
                for kp in range(NP):
                    nc.tensor.matmul(
                        ps, lhsT=wo_sb[:, 2 * kp:2 * kp + 2, t * 128:(t + 1) * 128],
                        rhs=ctx8[:, 2 * kp:2 * kp + 2, sl],
                        start=(kp == 0), stop=(kp == NP - 1), perf_mode=DR)
                # psum = WS*CS*(ctx@Wo); y8 = relu(psum*YS/(WS*CS) + YS*bo)
                nc.scalar.activation(yT8[:, t, sl], ps, AF.Relu,
                                     bias=bo8_sb[:, t:t + 1],
                                     scale=YS / (WS * CS))

        # ===== P6: r = sigmoid(x Wxr + y Wyr); rx = r * xT (bf16) ============
        with tc.tile_pool(name="p6", bufs=3) as p6:
            for t in range(NT):
                for c in range(2):
                    sl = slice(c * 512, (c + 1) * 512)
                    ps = psum.tile([128, 512], F32, tag="mm", bufs=2, name="ps_r")
                    for kt in range(NT):
                        nc.tensor.matmul(
                            ps, lhsT=wxr_sb[:, kt, t * 128:(t + 1) * 128],
                            rhs=xTb[:, kt, sl], start=(kt == 0), stop=False)
                    for kp in range(NP):
                        nc.tensor.matmul(
                            ps, lhsT=wyr_sb[:, 2 * kp:2 * kp + 2, t * 128:(t + 1) * 128],
                            rhs=yT8[:, 2 * kp:2 * kp + 2, sl],
                            start=False, stop=(kp == NP - 1), perf_mode=DR)
                    rt = p6.tile([128, 512], BF16, tag="rt")
                    nc.scalar.activation(rt, ps, AF.Sigmoid, scale=1.0 / WS)
                    nc.vector.tensor_mul(rx[:, t, sl], rt, xTb[:, t, sl])

        # =========== P7: z/h + gated combine (seq-major, single pass) ========
        with tc.tile_pool(name="p7", bufs=2) as p7:
            for st in range(NT):
                ss = slice(st * 128, (st + 1) * 128)
                xf = p7.tile([128, E], BF16, tag="xf")
                nc.sync.dma_start(out=xf, in_=xb_d.ap()[ss, :])
                ot = p7.tile([128, E], F32, tag="ot")
                for c in range(2):
                    sl = slice(c * 512, (c + 1) * 512)
                    zps = psum.tile([128, 512], F32, tag="mm", bufs=2, name="ps_z")
                    for kt in range(NT):
                        nc.tensor.matmul(zps, lhsT=xTb[:, kt, ss],
                                         rhs=wxz_sb[:, kt, sl],
                                         start=(kt == 0), stop=False)
                    for kp in range(NP):
                        nc.tensor.matmul(zps, lhsT=yT8[:, 2 * kp:2 * kp + 2, ss],
                                         rhs=wyz_sb[:, 2 * kp:2 * kp + 2, sl],
                                         start=False, stop=False, perf_mode=DR)
                    nc.tensor.matmul(zps, lhsT=ones1_b, rhs=bxz_row[:, sl],
                                     start=False, stop=True)
                    zt = p7.tile([128, 512], F32, tag="zt")
                    nc.scalar.activation(zt, zps, AF.Sigmoid, scale=1.0 / WS)
                    hps = psum.tile([128, 512], F32, tag="mm", bufs=2, name="ps_h")
                    for kt in range(NT):
                        nc.tensor.matmul(hps, lhsT=rx[:, kt, ss],
                                         rhs=wxg_sb[:, kt, sl],
                                         start=(kt == 0), stop=False)
                    for kp in range(NP):
                        nc.tensor.matmul(hps, lhsT=yT8[:, 2 * kp:2 * kp + 2, ss],
                                         rhs=wyg_sb[:, 2 * kp:2 * kp + 2, sl],
                                         start=False, stop=(kp == NP - 1), perf_mode=DR)
                    ht = p7.tile([128, 512], F32, tag="ht")
                    nc.scalar.activation(ht, hps, AF.Tanh, scale=1.0 / WS)
                    dt = p7.tile([128, 512], F32, tag="dt")
                    nc.gpsimd.tensor_sub(dt, ht, xf[:, sl])
                    zd = p7.tile([128, 512], F32, tag="zd")
                    nc.vector.tensor_mul(zd, zt, dt)
                    nc.vector.tensor_add(ot[:, sl], xf[:, sl], zd)
                nc.sync.dma_start(out=out_d.ap()[ss, :], in_=ot)

    nc.compile()
    return nc


# ---------------- host-side packing -----------------------------------------

def _pack_w(w, scale, npdt):
    return np.ascontiguousarray(
        (np.asarray(w, np.float32) * scale).astype(npdt)
        .reshape(NT, 128, E).transpose(1, 0, 2).reshape(128, NT * E))


def _pack_fm(m, npdt):
    # [E, S]-logical feature-major -> [128, NT*S]
    return np.ascontiguousarray(
        m.astype(npdt).reshape(NT, 128, S).transpose(1, 0, 2).reshape(128, NT * S))


def _pack_bias_fm(b, scale=1.0):
    return np.ascontiguousarray(
        (np.asarray(b, np.float32) * scale).reshape(NT, 128).T)


_INV = None


def _inv_pair():
    global _INV
    if _INV is None:
        inv = 1.0 / (10000.0 ** (np.arange(0, E, 2, dtype=np.float32) / np.float32(E)))
        _INV = np.repeat(inv.astype(np.float64), 2)  # pair-expanded [E]
    return _INV


def _tables(idx):
    f = _inv_pair()[:, None] * idx.astype(np.float64)[None, :]  # [E, S]
    sn = np.sin(f).astype(np.float32)
    sn[0::2, :] *= -1.0  # sign baked in for the pair-swap AP read
    return (_pack_fm(np.cos(f).astype(np.float32), NPBF16),
            _pack_fm(sn, NPBF16))


def make_in_maps(inputs, share_qk):
    x = np.asarray(inputs["x"], dtype=np.float32)
    qi = np.asarray(inputs["query_index"])
    ki = np.asarray(inputs["key_index"])
    common = {
        "bq": _pack_bias_fm(np.asarray(inputs["bq"])),
        "bk": _pack_bias_fm(np.asarray(inputs["bk"])),
        "bo8": _pack_bias_fm(np.asarray(inputs["bo"]), YS),
        "bv_row": (np.asarray(inputs["bv"], np.float32) * WS).astype(NPBF16).reshape(1, E),
        "bxz_row": (np.asarray(inputs["bxz"], np.float32) * WS).astype(NPBF16).reshape(1, E),
        "ones8": np.full((128, 256), 1.0 / CS, NPFP8),
    }
    for nm in ("Wq", "Wk", "Wv", "Wo"):
        common[nm] = _pack_w(inputs[nm], WS, NPFP8)
    for nm in ("Wyr", "Wyz", "Wyg"):
        common[nm] = _pack_w(inputs[nm], WS / YS, NPFP8)
    for nm in ("Wxr", "Wxz", "Wxg"):
        common[nm] = _pack_w(inputs[nm], WS, NPBF16)
    in_maps = []
    for b in range(B):
        m = dict(common)
        xb = np.ascontiguousarray(x[b])
        m["xb"] = xb.astype(NPBF16)
        xt = xb.T  # [E, S]
        m["xtb"] = _pack_fm(xt, NPBF16)
        m["xt8"] = _pack_fm(xt, NPFP8)
        m["cosq"], m["sinq"] = _tables(qi[b])
        if not share_qk:
            m["cosk"], m["sink"] = _tables(ki[b])
        in_maps.append(m)
    return in_maps


def kernel(**inputs):
    qi = np.asarray(inputs["query_index"])
    ki = np.asarray(inputs["key_index"])
    share_qk = bool(np.array_equal(qi, ki))

    key = ("k", share_qk)
    if key not in _COMPILED:
        _COMPILED[key] = _build(share_qk)
    nc = _COMPILED[key]

    in_maps = make_in_maps(inputs, share_qk)
    global _dbg_in_maps
    _dbg_in_maps = in_maps
    res = bass_utils.run_bass_kernel_spmd(nc, in_maps, core_ids=list(range(NC)))
    out = np.stack([res.results[b]["out"] for b in range(B)]).astype(np.float32)
    return out
